# revision 16
# baseline (speedup 1.0000x reference)
"""Single-head causal attention (B=8, T=2048, C=1024, H=64) on 8 TRN2 NeuronCores.

Strategy: data parallel (batch element b on core b) with a split-precision
fp8 pipeline. Per core, for its [T, C] slices q_b / k_b:

    Q = q_b @ Wq ; K = k_b @ Wk ; V = k_b @ Wv
    S = Q K^T / sqrt(C), causal ; P = exp(S) ; out = (P @ V) / (P @ 1)

Precision scheme (validated: max rel err ~6e-3 vs fp32 reference):
  * q, k stream in as fp8e4 (e4m3, max 240); weights are pre-scaled x32 and
    cast to fp8 so W entries (std 0.02) land in fp8's normal range.
  * Projections run fp8 DoubleRow (256-wide contraction chunks): the K and
    V projections share one pass ([Wk|Wv] stationary, 128 PSUM rows).
  * Scores are plain fp8 (contraction 64): S^T tiles [128 keys, n cols] so
    P^T feeds PV directly with no transposes. DoubleRow gains nothing here
    (the PE is moving-operand-ingest-bound) and would double the stream.
  * P and V are fp8 for out rows >= 256 (those average over >= 256 keys, so
    fp8 V quantization noise cancels ~1/sqrt(n)); rows 0:255 keep P, V in
    bf16 (few-key rows see V error directly). A separate bf16 copy of k's
    first 256 rows feeds the bf16 V-head projection.
  * PV uses fp8 DoubleRow with the pair dim = two key tiles: halves the
    P^T stream. V-natural tiles come from PE transposes of V^T (kept on
    SBUF partitions 64:128 straight from the fused KV projection PSUM,
    transposed from there via tile_position=(64,0)).
  * Softmax denominators come from a ones column in the PV stationary
    (32.0 in the fp8 path to match the 32-scaled V; the scale cancels in
    the final divide). The kernel emits UNNORMALIZED [65, T] (row 0 =
    denominator); the host does rows[1:65]/row[0] on unshard.
  * exp runs on the scalar engine out of PSUM in [128, 2, n] key-tile
    pairs; the activation table is pre-warmed with a dummy exp during the
    DMA fill. The scalar engine does nothing else (exp is ~21us).
  * Engine split: PE matmuls; Act exp; DVE proj/PSUM copies; Pool (gpsimd)
    diagonal masks, V18 zero fill, output stores. Inputs stream on the
    sync + scalar HWDGE rings (one tile per 512-col block for precise DMA
    deps); the slower gpsimd ring carries the k head + small consts.
  * Pipeline: a single global score->exp->PV chain over all key-tile
    pairs, with PV trailing scores by two pairs so the PE never waits on
    exp; projection blocks are interleaved at fixed points.
"""

import numpy as np
import ml_dtypes

B, T, C, H = 8, 2048, 1024, 64
P = 128
NB = T // 512             # 4 column blocks
NJ = T // P               # 16 key tiles
WS = 32.0                 # fp8 weight pre-scale
SCALE = float(C) ** -0.5 / (WS * WS)   # folded into the exp activation

_cached = {}


def _build():
    import concourse.bass as bass
    import concourse.mybir as mybir
    import concourse.tile as tile
    from concourse import bacc

    dt = mybir.dt
    DR = mybir.MatmulPerfMode.DoubleRow
    EXP = mybir.ActivationFunctionType.Exp
    nc = bacc.Bacc("TRN2", target_bir_lowering=False, debug=False, num_devices=B)

    # inputs (see _host_inputs for layouts)
    q8T = nc.dram_tensor("q8T", [NB, P, 4, 2, 512], dt.float8e4, kind="ExternalInput").ap()
    k8T = nc.dram_tensor("k8T", [NB, P, 4, 2, 512], dt.float8e4, kind="ExternalInput").ap()
    khT = nc.dram_tensor("khT", [P, 8, 256], dt.bfloat16, kind="ExternalInput").ap()
    wq8 = nc.dram_tensor("wq8", [P, 4, 2, H], dt.float8e4, kind="ExternalInput").ap()
    wkv8 = nc.dram_tensor("wkv8", [P, 4, 2, P], dt.float8e4, kind="ExternalInput").ap()
    wvh = nc.dram_tensor("wvh", [P, 8, H], dt.bfloat16, kind="ExternalInput").ap()
    mask8 = nc.dram_tensor("mask8", [P, 2, 256], dt.float8e4, kind="ExternalInput").ap()
    maskh = nc.dram_tensor("maskh", [P, 2, 256], dt.bfloat16, kind="ExternalInput").ap()
    idb8 = nc.dram_tensor("idb8", [H, H], dt.float8e4, kind="ExternalInput").ap()
    idbh = nc.dram_tensor("idbh", [H, H], dt.bfloat16, kind="ExternalInput").ap()
    out_t = nc.dram_tensor("out_t", [H + 1, T], dt.float32, kind="ExternalOutput").ap()

    with tile.TileContext(nc) as tc:
        with (
            tc.tile_pool(name="consts", bufs=1) as consts,
            tc.tile_pool(name="inbuf", bufs=1) as inbuf,
            tc.tile_pool(name="proj", bufs=1) as proj,
            tc.tile_pool(name="p8buf", bufs=4) as p8buf,
            tc.tile_pool(name="obuf", bufs=2) as obuf,
            tc.tile_pool(name="ppsum", bufs=1, space="PSUM") as ppsum,
            tc.tile_pool(name="vtpsum", bufs=1, space="PSUM") as vtpsum,
            tc.tile_pool(name="opsum", bufs=2, space="PSUM") as opsum,
            tc.tile_pool(name="spsum", bufs=2, space="PSUM") as spsum,
        ):
            wkv8_s = consts.tile([P, 4, 2, P], dt.float8e4)
            wq8_s = consts.tile([P, 4, 2, H], dt.float8e4)
            wvh_s = consts.tile([P, 8, H], dt.bfloat16)
            mask8_s = consts.tile([P, 2, 256], dt.float8e4)
            maskh_s = consts.tile([P, 2, 256], dt.bfloat16)
            idb8_s = consts.tile([P, H], dt.float8e4)  # identity on parts 64:128
            idbh_s = consts.tile([H, H], dt.bfloat16)

            QT8 = proj.tile([H, T], dt.float8e4)
            KT8 = proj.tile([H, T], dt.float8e4)
            VT8 = proj.tile([P, T], dt.float8e4)       # V^T on partitions 64:128
            VhT = proj.tile([H, 256], dt.bfloat16)
            V18 = proj.tile([P, NJ // 2, 2, P], dt.float8e4)   # ones(32)+V nat
            V1h = proj.tile([P, 2, 66], dt.bfloat16)           # ones(1)+Vhead
            Pth = proj.tile([P, 2, 256], dt.bfloat16)          # seg-0a P tiles

            # V18 zero fill (pad cols + unwritten regions) while DMAs run
            nc.gpsimd.memset(V18[:], 0.0)
            nc.gpsimd.memset(V18[:, :, :, 0:1], WS)
            nc.gpsimd.memset(V1h[:, :, 0:1], 1.0)

            # ---- input DMAs upfront: one tile per block for precise deps;
            # sync ring: k + small fp8 consts; scalar ring: q (then free
            # for exp); gpsimd ring (slow): k head + bf16 consts + stores.
            k8T_s = [inbuf.tile([P, 4, 2, 512], dt.float8e4, name=f"k8T{tb}")
                     for tb in range(NB)]
            q8T_s = [inbuf.tile([P, 4, 2, 512], dt.float8e4, name=f"q8T{tb}")
                     for tb in range(NB)]
            khT_s = inbuf.tile([P, 8, 256], dt.bfloat16)
            nc.scalar.dma_start(out=wq8_s[:], in_=wq8[:])
            nc.sync.dma_start(out=wkv8_s[:], in_=wkv8[:])
            for cc in range(4):
                nc.scalar.dma_start(out=q8T_s[0][:, cc], in_=q8T[0][:, cc])
                nc.sync.dma_start(out=k8T_s[0][:, cc], in_=k8T[0][:, cc])
            # warm the exp activation table while the fill runs
            scr = consts.tile([1, 2], dt.float32)
            nc.gpsimd.memset(scr[:], 0.0)
            scrd = consts.tile([1, 2], dt.bfloat16)
            nc.scalar.activation(out=scrd[:], in_=scr[:], func=EXP, scale=1.0)
            nc.gpsimd.dma_start(out=khT_s[:], in_=khT[:])
            nc.gpsimd.dma_start(out=wvh_s[:], in_=wvh[:])
            nc.gpsimd.dma_start(out=idbh_s[:], in_=idbh[:])
            nc.sync.dma_start(out=idb8_s[64:P, :], in_=idb8[:])
            nc.scalar.dma_start(out=q8T_s[1][:], in_=q8T[1])
            nc.sync.dma_start(out=k8T_s[1][:], in_=k8T[1])
            nc.sync.dma_start(out=maskh_s[:], in_=maskh[:])
            nc.scalar.dma_start(out=q8T_s[2][:], in_=q8T[2])
            nc.sync.dma_start(out=k8T_s[2][:], in_=k8T[2])
            nc.sync.dma_start(out=mask8_s[:], in_=mask8[:])
            nc.scalar.dma_start(out=q8T_s[3][:], in_=q8T[3])
            nc.sync.dma_start(out=k8T_s[3][:], in_=k8T[3])

            # ---- projection blocks ---------------------------------------
            def proj_block(tb):
                sl = slice(512 * tb, 512 * (tb + 1))
                QTp = ppsum.tile([H, 512], dt.float32, tag="pp")
                for cc in range(4):
                    nc.tensor.matmul(QTp[:], lhsT=wq8_s[:, cc], rhs=q8T_s[tb][:, cc],
                                     start=(cc == 0), stop=(cc == 3), perf_mode=DR)
                nc.vector.tensor_copy(out=QT8[:, sl], in_=QTp[:])
                KVp = ppsum.tile([P, 512], dt.float32, tag="pp")
                for cc in range(4):
                    nc.tensor.matmul(KVp[:], lhsT=wkv8_s[:, cc], rhs=k8T_s[tb][:, cc],
                                     start=(cc == 0), stop=(cc == 3), perf_mode=DR)
                nc.vector.tensor_copy(out=KT8[:, sl], in_=KVp[0:H, :])
                nc.vector.tensor_copy(out=VT8[H:P, sl], in_=KVp[H:P, :])
                for jj in range(4):
                    j = 4 * tb + jj
                    # fp8 PE transpose requires output element step of 2
                    vtp = vtpsum.tile([P, 2 * H], dt.float8e4, tag="vt")
                    nc.tensor.transpose(vtp[:, 0:2 * H:2],
                                        VT8[H:P, P * j:P * (j + 1)],
                                        idb8_s[H:P, :], tile_position=(H, 0))
                    nc.vector.tensor_copy(out=V18[:, j >> 1, j & 1, 1:65],
                                          in_=vtp[:, 0:2 * H:2])

            def vhead_block():
                VhTp = ppsum.tile([H, 256], dt.float32, tag="pp")
                for ch in range(8):
                    nc.tensor.matmul(VhTp[:], lhsT=wvh_s[:, ch], rhs=khT_s[:, ch],
                                     start=(ch == 0), stop=(ch == 7))
                nc.vector.tensor_copy(out=VhT[:], in_=VhTp[:])
                for j in range(2):
                    vtp = vtpsum.tile([P, H], dt.bfloat16, tag="vt")
                    nc.tensor.transpose(vtp[:], VhT[:, P * j:P * (j + 1)], idbh_s[:])
                    nc.vector.tensor_copy(out=V1h[:, j, 1:65], in_=vtp[:])

            # ---- attention work list -------------------------------------
            # segment = one OUT accumulation: (lo, w, path); its pairs:
            # (j0, n, off, diag) with off = col offset within the segment.
            segs = [(0, 256, "h", [(0, 256, 0, True)]),
                    (256, 256, "8", [(0, 256, 0, False), (2, 256, 0, True)])]
            for ic in range(1, NB):
                prs = [(2 * p, 512, 0, False) for p in range(2 * ic)]
                prs += [(4 * ic, 512, 0, True), (4 * ic + 2, 256, 256, True)]
                segs.append((512 * ic, 512, "8", prs))

            def scores(seg, pi):
                lo, w, path, prs = seg
                j0, n, off, diag = prs[pi]
                Sp = spsum.tile([P, 2, 512], dt.float32, tag="s")
                for kt in range(2):
                    j = j0 + kt
                    nc.tensor.matmul(Sp[:, kt, 0:n],
                                     lhsT=KT8[:, P * j:P * (j + 1)],
                                     rhs=QT8[:, lo + off:lo + off + n],
                                     start=True, stop=True)
                return Sp

            def exp_mask(seg, pi, Sp):
                lo, w, path, prs = seg
                j0, n, off, diag = prs[pi]
                if path == "h":
                    nc.scalar.activation(out=Pth[:, :, 0:n], in_=Sp[:, :, 0:n],
                                         func=EXP, scale=SCALE)
                    nc.gpsimd.tensor_mul(Pth[:, :, 0:256], Pth[:, :, 0:256],
                                         maskh_s[:])
                    return None
                Pt = p8buf.tile([P, 2, 512], dt.float8e4, tag="p8")
                nc.scalar.activation(out=Pt[:, :, 0:n], in_=Sp[:, :, 0:n],
                                     func=EXP, scale=SCALE)
                if diag:
                    nc.gpsimd.tensor_mul(Pt[:, :, 0:256], Pt[:, :, 0:256],
                                         mask8_s[:])
                return Pt

            outp = {}

            def pv_emit(si, pi, Pt):
                seg = segs[si]
                lo, w, path, prs = seg
                j0, n, off, diag = prs[pi]
                OUTp = outp[si]
                if path == "h":
                    for kt in range(2):
                        nc.tensor.matmul(OUTp[0:65, 0:256],
                                         lhsT=V1h[:, kt, 0:65],
                                         rhs=Pth[:, kt, 0:256],
                                         start=(kt == 0), stop=(kt == 1))
                else:
                    nc.tensor.matmul(OUTp[:, off:w], lhsT=V18[:, j0 >> 1, :, :],
                                     rhs=Pt[:, :, 0:n],
                                     start=(pi == 0), stop=(pi == len(prs) - 1),
                                     perf_mode=DR)
                if pi == len(prs) - 1:
                    ot = obuf.tile([H + 1, 512], dt.float32, tag="o",
                                   name=f"ot{si}")
                    nc.vector.tensor_copy(out=ot[:, 0:w], in_=OUTp[0:H + 1, 0:w])
                    nc.sync.dma_start(out=out_t[:, lo:lo + w], in_=ot[:, 0:w])

            work = [(si, pi) for si, seg in enumerate(segs)
                    for pi in range(len(seg[3]))]
            pending = []
            proj_hooks = {(1, 0): 1, (2, 1): 2, (3, 1): 3}
            proj_block(0)
            for si, pi in work:
                if pi == 0:
                    outp[si] = opsum.tile([P, 512], dt.float32, tag="out",
                                          name=f"OUTp{si}")
                Sp = scores(segs[si], pi)
                if (si, pi) in proj_hooks:
                    proj_block(proj_hooks[(si, pi)])
                if (si, pi) == (1, 1):
                    vhead_block()
                if len(pending) >= 3:
                    pv_emit(*pending.pop(0))
                pending.append((si, pi, exp_mask(segs[si], pi, Sp)))
            for args in pending:
                pv_emit(*args)

    nc.compile()
    return nc


def _get_nc():
    if "nc" not in _cached:
        _cached["nc"] = _build()
    return _cached["nc"]


def _block8(xT):
    """fp8 [C, T] -> [NB, P, 4, 2, 512]; c = cc*256 + kt*128 + p."""
    return np.ascontiguousarray(
        xT.reshape(4, 2, P, NB, 512).transpose(3, 2, 0, 1, 4))


def _w8(w):
    """fp8 [C, Hw] -> [P, 4, 2, Hw]."""
    return np.ascontiguousarray(
        w.reshape(4, 2, P, w.shape[1]).transpose(2, 0, 1, 3))


def _host_inputs(q, k, Wq, Wk, Wv):
    bf16 = ml_dtypes.bfloat16
    f8 = ml_dtypes.float8_e4m3
    wq8_h = _w8((WS * Wq).astype(f8))
    wkv8_h = _w8((WS * np.concatenate([Wk, Wv], axis=1)).astype(f8))
    wvh_h = np.ascontiguousarray(
        Wv.astype(bf16).reshape(8, P, H).transpose(1, 0, 2))
    tri = np.triu(np.ones((P, P), np.float32))
    m = np.zeros((P, 2, 256), np.float32)
    m[:, 0, 0:128] = tri
    m[:, 0, 128:256] = 1.0
    m[:, 1, 128:256] = tri
    idb = np.eye(H, dtype=np.float32)
    consts = {
        "wq8": wq8_h, "wkv8": wkv8_h, "wvh": wvh_h,
        "mask8": m.astype(f8), "maskh": m.astype(bf16),
        "idb8": idb.astype(f8), "idbh": idb.astype(bf16),
    }
    in_maps = []
    for b in range(B):
        in_maps.append({
            "q8T": _block8(q[b].T.astype(f8)),
            "k8T": _block8(k[b].T.astype(f8)),
            "khT": np.ascontiguousarray(
                k[b, :256].T.astype(bf16).reshape(8, P, 256).transpose(1, 0, 2)),
            **consts,
        })
    return in_maps


def _postprocess(res):
    out = np.empty((B, T, H), np.float32)
    for b in range(B):
        o = res.results[b]["out_t"]
        out[b] = (o[1:H + 1] / o[0:1]).T
    return out


def kernel(q, k, Wq, Wk, Wv):
    from concourse.bass_utils import run_bass_kernel_spmd

    nc = _get_nc()
    in_maps = _host_inputs(q, k, Wq, Wk, Wv)
    res = run_bass_kernel_spmd(nc, in_maps, list(range(B)))
    return _postprocess(res)


if __name__ == "__main__":
    rng = np.random.default_rng(0)
    q = rng.standard_normal((B, T, C)).astype(np.float32)
    k = rng.standard_normal((B, T, C)).astype(np.float32)
    Wq = (rng.standard_normal((C, H)) * 0.02).astype(np.float32)
    Wk = (rng.standard_normal((C, H)) * 0.02).astype(np.float32)
    Wv = (rng.standard_normal((C, H)) * 0.02).astype(np.float32)
    o = kernel(q, k, Wq, Wk, Wv)
    print("out", o.shape, o.dtype, float(np.abs(o).max()))


# revision 22
# speedup vs baseline: 1.1819x; 1.1819x over previous
"""Single-head causal attention (B=8, T=2048, C=1024, H=64) on 8 TRN2 NeuronCores.

Strategy: data parallel (batch element b on core b) with a split-precision
fp8 pipeline. Per core, for its [T, C] slices q_b / k_b:

    Q = q_b @ Wq ; K = k_b @ Wk ; V = k_b @ Wv
    S = Q K^T / sqrt(C), causal ; P = exp(S) ; out = (P @ V) / (P @ 1)

Precision scheme (validated: max rel err ~6e-3 vs fp32 reference):
  * q, k stream in as fp8e4 (e4m3, max 240); weights are pre-scaled x32 and
    cast to fp8 so W entries (std 0.02) land in fp8's normal range.
  * Projections run fp8 DoubleRow (256-wide contraction chunks): the K and
    V projections share one pass ([Wk|Wv] stationary, 128 PSUM rows).
  * Scores are plain fp8 (contraction 64): S^T tiles [128 keys, n cols] so
    P^T feeds PV directly with no transposes. DoubleRow gains nothing here
    (the PE is moving-operand-ingest-bound) and would double the stream.
  * P and V are fp8 for out rows >= 256 (those average over >= 256 keys, so
    fp8 V quantization noise cancels ~1/sqrt(n)); rows 0:255 keep P, V in
    bf16 (few-key rows see V error directly). A separate bf16 copy of k's
    first 256 rows feeds the bf16 V-head projection.
  * PV uses fp8 DoubleRow with the pair dim = two key tiles: halves the
    P^T stream. V-natural tiles come from PE transposes of V^T (kept on
    SBUF partitions 64:128 straight from the fused KV projection PSUM,
    transposed from there via tile_position=(64,0)).
  * Softmax denominators come from a ones column in the PV stationary
    (32.0 in the fp8 path to match the 32-scaled V; the scale cancels in
    the final divide). The kernel emits UNNORMALIZED [65, T] (row 0 =
    denominator); the host does rows[1:65]/row[0] on unshard.
  * exp runs on the scalar engine out of PSUM in [128, 2, n] key-tile
    pairs; the activation table is pre-warmed with a dummy exp during the
    DMA fill. The scalar engine does nothing else (exp is ~21us).
  * Engine split: PE matmuls; Act exp (plus the q-block DMA ring); DVE
    proj/PSUM copies and output staging; Pool (gpsimd) diagonal masks and
    V18 zero fill. Inputs stream on the sync + scalar HWDGE rings (one
    tile per 512-col block, blocks 0-1 split per 256-chunk, for precise
    DMA deps); the slower gpsimd ring carries the k head + bf16 consts;
    output stores ride the sync ring after the fill drains.
  * Pipeline: a single global score->exp->PV chain over all key-tile
    pairs, with PV trailing scores by three pairs so the PE never waits
    on exp or masks; projection blocks are interleaved at fixed points.
"""

import numpy as np
import ml_dtypes

B, T, C, H = 8, 2048, 1024, 64
P = 128
NB = T // 512             # 4 column blocks
NJ = T // P               # 16 key tiles
WS = 32.0                 # fp8 weight pre-scale
SCALE = float(C) ** -0.5 / (WS * WS)   # folded into the exp activation

_cached = {}


def _build():
    import concourse.bass as bass
    import concourse.mybir as mybir
    import concourse.tile as tile
    from concourse import bacc

    dt = mybir.dt
    DR = mybir.MatmulPerfMode.DoubleRow
    EXP = mybir.ActivationFunctionType.Exp
    nc = bacc.Bacc("TRN2", target_bir_lowering=False, debug=False, num_devices=B)

    # inputs (see _host_inputs for layouts)
    q8T = nc.dram_tensor("q8T", [NB, P, 4, 2, 512], dt.float8e4, kind="ExternalInput").ap()
    k8T = nc.dram_tensor("k8T", [NB, P, 4, 2, 512], dt.float8e4, kind="ExternalInput").ap()
    khT = nc.dram_tensor("khT", [P, 8, 256], dt.bfloat16, kind="ExternalInput").ap()
    wq8 = nc.dram_tensor("wq8", [P, 4, 2, H], dt.float8e4, kind="ExternalInput").ap()
    wkv8 = nc.dram_tensor("wkv8", [P, 4, 2, P], dt.float8e4, kind="ExternalInput").ap()
    wvh = nc.dram_tensor("wvh", [P, 8, H], dt.bfloat16, kind="ExternalInput").ap()
    mask8 = nc.dram_tensor("mask8", [P, 2, 256], dt.float8e4, kind="ExternalInput").ap()
    maskh = nc.dram_tensor("maskh", [P, 2, 256], dt.bfloat16, kind="ExternalInput").ap()
    idb8 = nc.dram_tensor("idb8", [H, H], dt.float8e4, kind="ExternalInput").ap()
    idbh = nc.dram_tensor("idbh", [H, H], dt.bfloat16, kind="ExternalInput").ap()
    out_t = nc.dram_tensor("out_t", [H + 1, T], dt.float32, kind="ExternalOutput").ap()

    with tile.TileContext(nc) as tc:
        with (
            tc.tile_pool(name="consts", bufs=1) as consts,
            tc.tile_pool(name="inbuf", bufs=1) as inbuf,
            tc.tile_pool(name="proj", bufs=1) as proj,
            tc.tile_pool(name="p8buf", bufs=4) as p8buf,
            tc.tile_pool(name="obuf", bufs=2) as obuf,
            tc.tile_pool(name="ppsum", bufs=2, space="PSUM") as ppsum,
            tc.tile_pool(name="vtpsum", bufs=1, space="PSUM") as vtpsum,
            tc.tile_pool(name="opsum", bufs=2, space="PSUM") as opsum,
            tc.tile_pool(name="spsum", bufs=3, space="PSUM") as spsum,
        ):
            wkv8_s = consts.tile([P, 4, 2, P], dt.float8e4)
            wq8_s = consts.tile([P, 4, 2, H], dt.float8e4)
            wvh_s = consts.tile([P, 8, H], dt.bfloat16)
            mask8_s = consts.tile([P, 2, 256], dt.float8e4)
            maskh_s = consts.tile([P, 2, 256], dt.bfloat16)
            idb8_s = consts.tile([P, H], dt.float8e4)  # identity on parts 64:128
            idbh_s = consts.tile([H, H], dt.bfloat16)

            QT8 = proj.tile([H, T], dt.float8e4)
            KT8 = proj.tile([H, T], dt.float8e4)
            VT8 = proj.tile([P, T], dt.float8e4)       # V^T on partitions 64:128
            VhT = proj.tile([H, 256], dt.bfloat16)
            V18 = proj.tile([P, NJ // 2, 2, P], dt.float8e4)   # ones(32)+V nat
            V1h = proj.tile([P, 2, 66], dt.bfloat16)           # ones(1)+Vhead
            Pth = proj.tile([P, 2, 256], dt.bfloat16)          # seg-0a P tiles

            # V18 zero fill (pad cols + unwritten regions) while DMAs run
            nc.gpsimd.memset(V18[:], 0.0)
            nc.gpsimd.memset(V18[:, :, :, 0:1], WS)
            nc.gpsimd.memset(V1h[:, :, 0:1], 1.0)

            # ---- input DMAs upfront: one tile per block for precise deps;
            # sync ring: k + small fp8 consts; scalar ring: q (then free
            # for exp); gpsimd ring (slow): k head + bf16 consts + stores.
            k8T_s = [inbuf.tile([P, 4, 2, 512], dt.float8e4, name=f"k8T{tb}")
                     for tb in range(NB)]
            q8T_s = [inbuf.tile([P, 4, 2, 512], dt.float8e4, name=f"q8T{tb}")
                     for tb in range(NB)]
            khT_s = inbuf.tile([P, 8, 256], dt.bfloat16)
            nc.scalar.dma_start(out=wq8_s[:], in_=wq8[:])
            nc.sync.dma_start(out=wkv8_s[:], in_=wkv8[:])
            for cc in range(4):
                nc.scalar.dma_start(out=q8T_s[0][:, cc], in_=q8T[0][:, cc])
                nc.sync.dma_start(out=k8T_s[0][:, cc], in_=k8T[0][:, cc])
            # warm the exp activation table while the fill runs
            scr = consts.tile([1, 2], dt.float32)
            nc.gpsimd.memset(scr[:], 0.0)
            scrd = consts.tile([1, 2], dt.bfloat16)
            nc.scalar.activation(out=scrd[:], in_=scr[:], func=EXP, scale=1.0)
            nc.gpsimd.dma_start(out=khT_s[:], in_=khT[:])
            nc.gpsimd.dma_start(out=wvh_s[:], in_=wvh[:])
            nc.gpsimd.dma_start(out=idbh_s[:], in_=idbh[:])
            nc.sync.dma_start(out=idb8_s[64:P, :], in_=idb8[:])
            for cc in range(4):
                nc.scalar.dma_start(out=q8T_s[1][:, cc], in_=q8T[1][:, cc])
                nc.sync.dma_start(out=k8T_s[1][:, cc], in_=k8T[1][:, cc])
            nc.sync.dma_start(out=maskh_s[:], in_=maskh[:])
            nc.scalar.dma_start(out=q8T_s[2][:], in_=q8T[2])
            nc.sync.dma_start(out=k8T_s[2][:], in_=k8T[2])
            nc.sync.dma_start(out=mask8_s[:], in_=mask8[:])
            nc.scalar.dma_start(out=q8T_s[3][:], in_=q8T[3])
            nc.sync.dma_start(out=k8T_s[3][:], in_=k8T[3])

            # ---- projection blocks ---------------------------------------
            def proj_block(tb):
                sl = slice(512 * tb, 512 * (tb + 1))
                QTp = ppsum.tile([H, 512], dt.float32, tag="pp")
                for cc in range(4):
                    nc.tensor.matmul(QTp[:], lhsT=wq8_s[:, cc], rhs=q8T_s[tb][:, cc],
                                     start=(cc == 0), stop=(cc == 3), perf_mode=DR)
                nc.vector.tensor_copy(out=QT8[:, sl], in_=QTp[:])
                KVp = ppsum.tile([P, 512], dt.float32, tag="pp")
                for cc in range(4):
                    nc.tensor.matmul(KVp[:], lhsT=wkv8_s[:, cc], rhs=k8T_s[tb][:, cc],
                                     start=(cc == 0), stop=(cc == 3), perf_mode=DR)
                nc.vector.tensor_copy(out=KT8[:, sl], in_=KVp[0:H, :])
                nc.vector.tensor_copy(out=VT8[H:P, sl], in_=KVp[H:P, :])
                for jj in range(4):
                    j = 4 * tb + jj
                    # fp8 PE transpose requires output element step of 2
                    vtp = vtpsum.tile([P, 2 * H], dt.float8e4, tag="vt")
                    nc.tensor.transpose(vtp[:, 0:2 * H:2],
                                        VT8[H:P, P * j:P * (j + 1)],
                                        idb8_s[H:P, :], tile_position=(H, 0))
                    nc.vector.tensor_copy(out=V18[:, j >> 1, j & 1, 1:65],
                                          in_=vtp[:, 0:2 * H:2])

            def vhead_block():
                VhTp = ppsum.tile([H, 256], dt.float32, tag="pp")
                for ch in range(8):
                    nc.tensor.matmul(VhTp[:], lhsT=wvh_s[:, ch], rhs=khT_s[:, ch],
                                     start=(ch == 0), stop=(ch == 7))
                nc.vector.tensor_copy(out=VhT[:], in_=VhTp[:])
                for j in range(2):
                    vtp = vtpsum.tile([P, H], dt.bfloat16, tag="vt")
                    nc.tensor.transpose(vtp[:], VhT[:, P * j:P * (j + 1)], idbh_s[:])
                    nc.vector.tensor_copy(out=V1h[:, j, 1:65], in_=vtp[:])

            # ---- attention work list -------------------------------------
            # segment = one OUT accumulation: (lo, w, path); its pairs:
            # (j0, n, off, diag) with off = col offset within the segment.
            segs = [(0, 256, "h", [(0, 256, 0, True)]),
                    (256, 256, "8", [(0, 256, 0, False), (2, 256, 0, True)])]
            for ic in range(1, NB):
                prs = [(2 * p, 512, 0, False) for p in range(2 * ic)]
                prs += [(4 * ic, 512, 0, True), (4 * ic + 2, 256, 256, True)]
                segs.append((512 * ic, 512, "8", prs))

            def scores(seg, pi):
                lo, w, path, prs = seg
                j0, n, off, diag = prs[pi]
                Sp = []
                for kt in range(2):
                    j = j0 + kt
                    S1 = spsum.tile([P, 512], dt.float32, tag="s",
                                    name=f"S{j}")
                    nc.tensor.matmul(S1[:, 0:n],
                                     lhsT=KT8[:, P * j:P * (j + 1)],
                                     rhs=QT8[:, lo + off:lo + off + n],
                                     start=True, stop=True)
                    Sp.append(S1)
                return Sp

            def exp_mask(seg, pi, Sp):
                lo, w, path, prs = seg
                j0, n, off, diag = prs[pi]
                if path == "h":
                    for kt in range(2):
                        nc.scalar.activation(out=Pth[:, kt, 0:n],
                                             in_=Sp[kt][:, 0:n],
                                             func=EXP, scale=SCALE)
                    nc.gpsimd.tensor_mul(Pth[:, :, 0:256], Pth[:, :, 0:256],
                                         maskh_s[:])
                    return None
                Pt = p8buf.tile([P, 2, 512], dt.float8e4, tag="p8")
                for kt in range(2):
                    nc.scalar.activation(out=Pt[:, kt, 0:n], in_=Sp[kt][:, 0:n],
                                         func=EXP, scale=SCALE)
                if diag:
                    nc.gpsimd.tensor_mul(Pt[:, :, 0:256], Pt[:, :, 0:256],
                                         mask8_s[:])
                return Pt

            outp = {}

            def pv_emit(si, pi, Pt):
                seg = segs[si]
                lo, w, path, prs = seg
                j0, n, off, diag = prs[pi]
                OUTp = outp[si]
                if path == "h":
                    for kt in range(2):
                        nc.tensor.matmul(OUTp[0:65, 0:256],
                                         lhsT=V1h[:, kt, 0:65],
                                         rhs=Pth[:, kt, 0:256],
                                         start=(kt == 0), stop=(kt == 1))
                else:
                    nc.tensor.matmul(OUTp[:, off:w], lhsT=V18[:, j0 >> 1, :, :],
                                     rhs=Pt[:, :, 0:n],
                                     start=(pi == 0), stop=(pi == len(prs) - 1),
                                     perf_mode=DR)
                if pi == len(prs) - 1:
                    ot = obuf.tile([H + 1, 512], dt.float32, tag="o",
                                   name=f"ot{si}")
                    nc.vector.tensor_copy(out=ot[:, 0:w], in_=OUTp[0:H + 1, 0:w])
                    nc.sync.dma_start(out=out_t[:, lo:lo + w], in_=ot[:, 0:w])

            work = [(si, pi) for si, seg in enumerate(segs)
                    for pi in range(len(seg[3]))]
            pending = []
            proj_hooks = {(1, 0): 1, (2, 1): 2, (3, 1): 3}
            proj_block(0)
            for si, pi in work:
                if pi == 0:
                    outp[si] = opsum.tile([P, 512], dt.float32, tag="out",
                                          name=f"OUTp{si}")
                Sp = scores(segs[si], pi)
                if (si, pi) in proj_hooks:
                    proj_block(proj_hooks[(si, pi)])
                if (si, pi) == (1, 1):
                    vhead_block()
                if len(pending) >= 3:
                    pv_emit(*pending.pop(0))
                pending.append((si, pi, exp_mask(segs[si], pi, Sp)))
            for args in pending:
                pv_emit(*args)

    nc.compile()
    return nc


def _get_nc():
    if "nc" not in _cached:
        _cached["nc"] = _build()
    return _cached["nc"]


def _block8(xT):
    """fp8 [C, T] -> [NB, P, 4, 2, 512]; c = cc*256 + kt*128 + p."""
    return np.ascontiguousarray(
        xT.reshape(4, 2, P, NB, 512).transpose(3, 2, 0, 1, 4))


def _w8(w):
    """fp8 [C, Hw] -> [P, 4, 2, Hw]."""
    return np.ascontiguousarray(
        w.reshape(4, 2, P, w.shape[1]).transpose(2, 0, 1, 3))


def _host_inputs(q, k, Wq, Wk, Wv):
    bf16 = ml_dtypes.bfloat16
    f8 = ml_dtypes.float8_e4m3
    wq8_h = _w8((WS * Wq).astype(f8))
    wkv8_h = _w8((WS * np.concatenate([Wk, Wv], axis=1)).astype(f8))
    wvh_h = np.ascontiguousarray(
        Wv.astype(bf16).reshape(8, P, H).transpose(1, 0, 2))
    tri = np.triu(np.ones((P, P), np.float32))
    m = np.zeros((P, 2, 256), np.float32)
    m[:, 0, 0:128] = tri
    m[:, 0, 128:256] = 1.0
    m[:, 1, 128:256] = tri
    idb = np.eye(H, dtype=np.float32)
    consts = {
        "wq8": wq8_h, "wkv8": wkv8_h, "wvh": wvh_h,
        "mask8": m.astype(f8), "maskh": m.astype(bf16),
        "idb8": idb.astype(f8), "idbh": idb.astype(bf16),
    }
    in_maps = []
    for b in range(B):
        in_maps.append({
            "q8T": _block8(q[b].T.astype(f8)),
            "k8T": _block8(k[b].T.astype(f8)),
            "khT": np.ascontiguousarray(
                k[b, :256].T.astype(bf16).reshape(8, P, 256).transpose(1, 0, 2)),
            **consts,
        })
    return in_maps


def _postprocess(res):
    out = np.empty((B, T, H), np.float32)
    for b in range(B):
        o = res.results[b]["out_t"]
        out[b] = (o[1:H + 1] / o[0:1]).T
    return out


def kernel(q, k, Wq, Wk, Wv):
    from concourse.bass_utils import run_bass_kernel_spmd

    nc = _get_nc()
    in_maps = _host_inputs(q, k, Wq, Wk, Wv)
    res = run_bass_kernel_spmd(nc, in_maps, list(range(B)))
    return _postprocess(res)


if __name__ == "__main__":
    rng = np.random.default_rng(0)
    q = rng.standard_normal((B, T, C)).astype(np.float32)
    k = rng.standard_normal((B, T, C)).astype(np.float32)
    Wq = (rng.standard_normal((C, H)) * 0.02).astype(np.float32)
    Wk = (rng.standard_normal((C, H)) * 0.02).astype(np.float32)
    Wv = (rng.standard_normal((C, H)) * 0.02).astype(np.float32)
    o = kernel(q, k, Wq, Wk, Wv)
    print("out", o.shape, o.dtype, float(np.abs(o).max()))


# revision 24
# speedup vs baseline: 1.2091x; 1.0230x over previous
"""Single-head causal attention (B=8, T=2048, C=1024, H=64) on 8 TRN2 NeuronCores.

Strategy: data parallel (batch element b on core b) with a split-precision
fp8 pipeline. Per core, for its [T, C] slices q_b / k_b:

    Q = q_b @ Wq ; K = k_b @ Wk ; V = k_b @ Wv
    S = Q K^T / sqrt(C), causal ; P = exp(S) ; out = (P @ V) / (P @ 1)

Precision scheme (validated: max rel err ~6e-3 vs fp32 reference):
  * q, k stream in as fp8e4 (e4m3, max 240); weights are pre-scaled x32 and
    cast to fp8 so W entries (std 0.02) land in fp8's normal range.
  * Projections run fp8 DoubleRow (256-wide contraction chunks): the K and
    V projections share one pass ([Wk|Wv] stationary, 128 PSUM rows).
  * Scores are plain fp8 (contraction 64): S^T tiles [128 keys, n cols] so
    P^T feeds PV directly with no transposes. DoubleRow gains nothing here
    (the PE is moving-operand-ingest-bound) and would double the stream.
  * P and V are fp8 for out rows >= 256 (those average over >= 256 keys, so
    fp8 V quantization noise cancels ~1/sqrt(n)); rows 0:255 keep P, V in
    bf16 (few-key rows see V error directly). A separate bf16 copy of k's
    first 256 rows feeds the bf16 V-head projection.
  * PV uses fp8 DoubleRow with the pair dim = two key tiles: halves the
    P^T stream. V-natural tiles come from PE transposes of V^T (kept on
    SBUF partitions 64:128 straight from the fused KV projection PSUM,
    transposed from there via tile_position=(64,0)).
  * Softmax denominators come from a ones column in the PV stationary
    (32.0 in the fp8 path to match the 32-scaled V; the scale cancels in
    the final divide). The kernel emits UNNORMALIZED [65, T] (row 0 =
    denominator); the host does rows[1:65]/row[0] on unshard.
  * exp runs on the scalar engine out of PSUM in [128, 2, n] key-tile
    pairs; the activation table is pre-warmed with a dummy exp during the
    DMA fill. The scalar engine does nothing else (exp is ~21us).
  * Engine split: PE matmuls; Act exp (plus the q-block DMA ring); DVE
    proj/PSUM copies and output staging; Pool (gpsimd) diagonal masks and
    V18 zero fill. Inputs stream on the sync + scalar HWDGE rings (one
    tile per 512-col block, blocks 0-1 split per 256-chunk, for precise
    DMA deps); the slower gpsimd ring carries the k head + bf16 consts;
    output stores ride the sync ring after the fill drains.
  * Pipeline: a single global score->exp->PV chain over all key-tile
    pairs, with PV trailing scores by three pairs so the PE never waits
    on exp or masks; projection blocks are interleaved at fixed points.
"""

import numpy as np
import ml_dtypes

B, T, C, H = 8, 2048, 1024, 64
P = 128
NB = T // 512             # 4 column blocks
NJ = T // P               # 16 key tiles
WS = 32.0                 # fp8 weight pre-scale
SCALE = float(C) ** -0.5 / (WS * WS)   # folded into the exp activation

_cached = {}


def _build():
    import concourse.bass as bass
    import concourse.mybir as mybir
    import concourse.tile as tile
    from concourse import bacc

    dt = mybir.dt
    DR = mybir.MatmulPerfMode.DoubleRow
    EXP = mybir.ActivationFunctionType.Exp
    nc = bacc.Bacc("TRN2", target_bir_lowering=False, debug=False, num_devices=B)

    # inputs (see _host_inputs for layouts)
    q8T = nc.dram_tensor("q8T", [NB, P, 4, 2, 512], dt.float8e4, kind="ExternalInput").ap()
    k8T = nc.dram_tensor("k8T", [NB, P, 4, 2, 512], dt.float8e4, kind="ExternalInput").ap()
    khT = nc.dram_tensor("khT", [P, 8, 256], dt.bfloat16, kind="ExternalInput").ap()
    wq8 = nc.dram_tensor("wq8", [P, 4, 2, H], dt.float8e4, kind="ExternalInput").ap()
    wkv8 = nc.dram_tensor("wkv8", [P, 4, 2, P], dt.float8e4, kind="ExternalInput").ap()
    wvh = nc.dram_tensor("wvh", [P, 8, H], dt.bfloat16, kind="ExternalInput").ap()
    mask8 = nc.dram_tensor("mask8", [P, 2, 256], dt.float8e4, kind="ExternalInput").ap()
    maskh = nc.dram_tensor("maskh", [P, 2, 256], dt.bfloat16, kind="ExternalInput").ap()
    idb8 = nc.dram_tensor("idb8", [H, H], dt.float8e4, kind="ExternalInput").ap()
    idbh = nc.dram_tensor("idbh", [H, H], dt.bfloat16, kind="ExternalInput").ap()
    out_t = nc.dram_tensor("out_t", [H + 1, T], dt.float32, kind="ExternalOutput").ap()

    with tile.TileContext(nc) as tc:
        with (
            tc.tile_pool(name="consts", bufs=1) as consts,
            tc.tile_pool(name="inbuf", bufs=1) as inbuf,
            tc.tile_pool(name="proj", bufs=1) as proj,
            tc.tile_pool(name="p8buf", bufs=4) as p8buf,
            tc.tile_pool(name="obuf", bufs=2) as obuf,
            tc.tile_pool(name="ppsum", bufs=2, space="PSUM") as ppsum,
            tc.tile_pool(name="vtpsum", bufs=1, space="PSUM") as vtpsum,
            tc.tile_pool(name="opsum", bufs=2, space="PSUM") as opsum,
            tc.tile_pool(name="spsum", bufs=3, space="PSUM") as spsum,
        ):
            wkv8_s = consts.tile([P, 4, 2, P], dt.float8e4)
            wq8_s = consts.tile([P, 4, 2, H], dt.float8e4)
            wvh_s = consts.tile([P, 8, H], dt.bfloat16)
            mask8_s = consts.tile([P, 2, 256], dt.float8e4)
            maskh_s = consts.tile([P, 2, 256], dt.bfloat16)
            idb8_s = consts.tile([P, H], dt.float8e4)  # identity on parts 64:128
            idbh_s = consts.tile([H, H], dt.bfloat16)

            QT8 = proj.tile([H, T], dt.float8e4)
            KT8 = proj.tile([H, T], dt.float8e4)
            VT8 = proj.tile([P, T], dt.float8e4)       # V^T on partitions 64:128
            VhT = proj.tile([H, 256], dt.bfloat16)
            V18 = proj.tile([P, NJ // 2, 2, P], dt.float8e4)   # ones(32)+V nat
            V1h = proj.tile([P, 2, 66], dt.bfloat16)           # ones(1)+Vhead
            Pth = proj.tile([P, 2, 256], dt.bfloat16)          # seg-0a P tiles

            # V18 zero fill (pad cols + unwritten regions) while DMAs run
            nc.gpsimd.memset(V18[:], 0.0)
            nc.gpsimd.memset(V18[:, :, :, 0:1], WS)
            nc.gpsimd.memset(V1h[:, :, 0:1], 1.0)

            # ---- input DMAs upfront: one tile per block for precise deps;
            # sync ring: k + small fp8 consts; scalar ring: q (then free
            # for exp); gpsimd ring (slow): k head + bf16 consts + stores.
            k8T_s = [inbuf.tile([P, 4, 2, 512], dt.float8e4, name=f"k8T{tb}")
                     for tb in range(NB)]
            q8T_s = [inbuf.tile([P, 4, 2, 512], dt.float8e4, name=f"q8T{tb}")
                     for tb in range(NB)]
            khT_s = inbuf.tile([P, 8, 256], dt.bfloat16)
            nc.scalar.dma_start(out=wq8_s[:], in_=wq8[:])
            nc.sync.dma_start(out=wkv8_s[:], in_=wkv8[:])
            for cc in range(4):
                nc.scalar.dma_start(out=q8T_s[0][:, cc], in_=q8T[0][:, cc])
                nc.sync.dma_start(out=k8T_s[0][:, cc], in_=k8T[0][:, cc])
            # warm the exp activation table while the fill runs
            scr = consts.tile([1, 2], dt.float32)
            nc.gpsimd.memset(scr[:], 0.0)
            scrd = consts.tile([1, 2], dt.bfloat16)
            nc.scalar.activation(out=scrd[:], in_=scr[:], func=EXP, scale=1.0)
            nc.gpsimd.dma_start(out=khT_s[:], in_=khT[:])
            nc.gpsimd.dma_start(out=wvh_s[:], in_=wvh[:])
            nc.gpsimd.dma_start(out=idbh_s[:], in_=idbh[:])
            nc.sync.dma_start(out=idb8_s[64:P, :], in_=idb8[:])
            for cc in range(4):
                nc.scalar.dma_start(out=q8T_s[1][:, cc], in_=q8T[1][:, cc])
                nc.sync.dma_start(out=k8T_s[1][:, cc], in_=k8T[1][:, cc])
            nc.sync.dma_start(out=maskh_s[:], in_=maskh[:])
            nc.scalar.dma_start(out=q8T_s[2][:], in_=q8T[2])
            nc.sync.dma_start(out=k8T_s[2][:], in_=k8T[2])
            nc.sync.dma_start(out=mask8_s[:], in_=mask8[:])
            nc.scalar.dma_start(out=q8T_s[3][:], in_=q8T[3])
            nc.sync.dma_start(out=k8T_s[3][:], in_=k8T[3])

            # ---- projection blocks ---------------------------------------
            def proj_block(tb):
                sl = slice(512 * tb, 512 * (tb + 1))
                QTp = ppsum.tile([H, 512], dt.float32, tag="pp")
                for cc in range(4):
                    nc.tensor.matmul(QTp[:], lhsT=wq8_s[:, cc], rhs=q8T_s[tb][:, cc],
                                     start=(cc == 0), stop=(cc == 3), perf_mode=DR)
                nc.vector.tensor_copy(out=QT8[:, sl], in_=QTp[:])
                KVp = ppsum.tile([P, 512], dt.float32, tag="pp")
                for cc in range(4):
                    nc.tensor.matmul(KVp[:], lhsT=wkv8_s[:, cc], rhs=k8T_s[tb][:, cc],
                                     start=(cc == 0), stop=(cc == 3), perf_mode=DR)
                nc.vector.tensor_copy(out=KT8[:, sl], in_=KVp[0:H, :])
                nc.vector.tensor_copy(out=VT8[H:P, sl], in_=KVp[H:P, :])
                for jj in range(4):
                    j = 4 * tb + jj
                    # fp8 PE transpose requires output element step of 2
                    vtp = vtpsum.tile([P, 2 * H], dt.float8e4, tag="vt")
                    nc.tensor.transpose(vtp[:, 0:2 * H:2],
                                        VT8[H:P, P * j:P * (j + 1)],
                                        idb8_s[H:P, :], tile_position=(H, 0))
                    nc.vector.tensor_copy(out=V18[:, j >> 1, j & 1, 1:65],
                                          in_=vtp[:, 0:2 * H:2])

            def vhead_block():
                VhTp = ppsum.tile([H, 256], dt.float32, tag="pp")
                for ch in range(8):
                    nc.tensor.matmul(VhTp[:], lhsT=wvh_s[:, ch], rhs=khT_s[:, ch],
                                     start=(ch == 0), stop=(ch == 7))
                nc.vector.tensor_copy(out=VhT[:], in_=VhTp[:])
                for j in range(2):
                    vtp = vtpsum.tile([P, H], dt.bfloat16, tag="vt")
                    nc.tensor.transpose(vtp[:], VhT[:, P * j:P * (j + 1)], idbh_s[:])
                    nc.vector.tensor_copy(out=V1h[:, j, 1:65], in_=vtp[:])

            # ---- attention work list -------------------------------------
            # segment = one OUT accumulation: (lo, w, path); its pairs:
            # (j0, n, off, diag) with off = col offset within the segment.
            segs = [(0, 256, "h", [(0, 256, 0, True)]),
                    (256, 256, "8", [(0, 256, 0, False), (2, 256, 0, True)])]
            for ic in range(1, NB):
                prs = [(2 * p, 512, 0, False) for p in range(2 * ic)]
                prs += [(4 * ic, 512, 0, True), (4 * ic + 2, 256, 256, True)]
                segs.append((512 * ic, 512, "8", prs))

            def scores(seg, pi):
                lo, w, path, prs = seg
                j0, n, off, diag = prs[pi]
                Sp = []
                for kt in range(2):
                    j = j0 + kt
                    S1 = spsum.tile([P, 512], dt.float32, tag="s",
                                    name=f"S{j}")
                    nc.tensor.matmul(S1[:, 0:n],
                                     lhsT=KT8[:, P * j:P * (j + 1)],
                                     rhs=QT8[:, lo + off:lo + off + n],
                                     start=True, stop=True)
                    Sp.append(S1)
                return Sp

            def exp_mask(seg, pi, Sp):
                lo, w, path, prs = seg
                j0, n, off, diag = prs[pi]
                if path == "h":
                    for kt in range(2):
                        nc.scalar.activation(out=Pth[:, kt, 0:n],
                                             in_=Sp[kt][:, 0:n],
                                             func=EXP, scale=SCALE)
                    nc.gpsimd.tensor_mul(Pth[:, :, 0:256], Pth[:, :, 0:256],
                                         maskh_s[:])
                    return None
                Pt = p8buf.tile([P, 2, 512], dt.float8e4, tag="p8")
                for kt in range(2):
                    nc.scalar.activation(out=Pt[:, kt, 0:n], in_=Sp[kt][:, 0:n],
                                         func=EXP, scale=SCALE)
                if diag:
                    nc.gpsimd.tensor_mul(Pt[:, :, 0:256], Pt[:, :, 0:256],
                                         mask8_s[:])
                return Pt

            outp = {}

            def pv_emit(si, pi, Pt):
                seg = segs[si]
                lo, w, path, prs = seg
                j0, n, off, diag = prs[pi]
                OUTp = outp[si]
                if path == "h":
                    for kt in range(2):
                        nc.tensor.matmul(OUTp[0:65, 0:256],
                                         lhsT=V1h[:, kt, 0:65],
                                         rhs=Pth[:, kt, 0:256],
                                         start=(kt == 0), stop=(kt == 1))
                else:
                    nc.tensor.matmul(OUTp[:, off:w], lhsT=V18[:, j0 >> 1, :, :],
                                     rhs=Pt[:, :, 0:n],
                                     start=(pi == 0), stop=(pi == len(prs) - 1),
                                     perf_mode=DR)
                if pi == len(prs) - 1:
                    ot = obuf.tile([H + 1, 512], dt.float32, tag="o",
                                   name=f"ot{si}")
                    nc.vector.tensor_copy(out=ot[:, 0:w], in_=OUTp[0:H + 1, 0:w])
                    nc.sync.dma_start(out=out_t[:, lo:lo + w], in_=ot[:, 0:w])

            work = [(si, pi) for si, seg in enumerate(segs)
                    for pi in range(len(seg[3]))]
            pending = []
            proj_hooks = {(1, 0): 1, (2, 1): 2, (3, 1): 3}
            proj_block(0)
            for si, pi in work:
                if pi == 0:
                    outp[si] = opsum.tile([P, 512], dt.float32, tag="out",
                                          name=f"OUTp{si}")
                Sp = scores(segs[si], pi)
                if (si, pi) in proj_hooks:
                    proj_block(proj_hooks[(si, pi)])
                if (si, pi) == (1, 1):
                    vhead_block()
                if len(pending) >= 3:
                    pv_emit(*pending.pop(0))
                pending.append((si, pi, exp_mask(segs[si], pi, Sp)))
            for args in pending:
                pv_emit(*args)

    nc.compile()
    return nc


def _get_nc():
    if "nc" not in _cached:
        _cached["nc"] = _build()
    return _cached["nc"]


def _block8(xT):
    """fp8 [C, T] -> [NB, P, 4, 2, 512]; c = cc*256 + kt*128 + p."""
    return np.ascontiguousarray(
        xT.reshape(4, 2, P, NB, 512).transpose(3, 2, 0, 1, 4))


def _w8(w):
    """fp8 [C, Hw] -> [P, 4, 2, Hw]."""
    return np.ascontiguousarray(
        w.reshape(4, 2, P, w.shape[1]).transpose(2, 0, 1, 3))


def _host_inputs(q, k, Wq, Wk, Wv):
    bf16 = ml_dtypes.bfloat16
    f8 = ml_dtypes.float8_e4m3
    wq8_h = _w8((WS * Wq).astype(f8))
    wkv8_h = _w8((WS * np.concatenate([Wk, Wv], axis=1)).astype(f8))
    wvh_h = np.ascontiguousarray(
        Wv.astype(bf16).reshape(8, P, H).transpose(1, 0, 2))
    tri = np.triu(np.ones((P, P), np.float32))
    m = np.zeros((P, 2, 256), np.float32)
    m[:, 0, 0:128] = tri
    m[:, 0, 128:256] = 1.0
    m[:, 1, 128:256] = tri
    idb = np.eye(H, dtype=np.float32)
    consts = {
        "wq8": wq8_h, "wkv8": wkv8_h, "wvh": wvh_h,
        "mask8": m.astype(f8), "maskh": m.astype(bf16),
        "idb8": idb.astype(f8), "idbh": idb.astype(bf16),
    }
    in_maps = []
    for b in range(B):
        in_maps.append({
            "q8T": _block8(q[b].T.astype(f8)),
            "k8T": _block8(k[b].T.astype(f8)),
            "khT": np.ascontiguousarray(
                k[b, :256].T.astype(bf16).reshape(8, P, 256).transpose(1, 0, 2)),
            **consts,
        })
    return in_maps


def _postprocess(res):
    out = np.empty((B, T, H), np.float32)
    for b in range(B):
        o = res.results[b]["out_t"]
        out[b] = (o[1:H + 1] / o[0:1]).T
    return out


def kernel(q, k, Wq, Wk, Wv):
    from concourse.bass_utils import run_bass_kernel_spmd

    nc = _get_nc()
    in_maps = _host_inputs(q, k, Wq, Wk, Wv)
    res = run_bass_kernel_spmd(nc, in_maps, list(range(B)))
    return _postprocess(res)


if __name__ == "__main__":
    rng = np.random.default_rng(0)
    q = rng.standard_normal((B, T, C)).astype(np.float32)
    k = rng.standard_normal((B, T, C)).astype(np.float32)
    Wq = (rng.standard_normal((C, H)) * 0.02).astype(np.float32)
    Wk = (rng.standard_normal((C, H)) * 0.02).astype(np.float32)
    Wv = (rng.standard_normal((C, H)) * 0.02).astype(np.float32)
    o = kernel(q, k, Wq, Wk, Wv)
    print("out", o.shape, o.dtype, float(np.abs(o).max()))


# revision 28
# speedup vs baseline: 1.2435x; 1.0285x over previous
"""Single-head causal attention (B=8, T=2048, C=1024, H=64) on 8 TRN2 NeuronCores.

Strategy: data parallel (batch element b on core b) with a split-precision
fp8 pipeline. Per core, for its [T, C] slices q_b / k_b:

    Q = q_b @ Wq ; K = k_b @ Wk ; V = k_b @ Wv
    S = Q K^T / sqrt(C), causal ; P = exp(S) ; out = (P @ V) / (P @ 1)

Precision scheme (validated: max rel err ~6e-3 vs fp32 reference):
  * q, k stream in as fp8e4 (e4m3, max 240); weights are pre-scaled x32 and
    cast to fp8 so W entries (std 0.02) land in fp8's normal range.
  * Projections run fp8 DoubleRow (256-wide contraction chunks): the K and
    V projections share one pass ([Wk|Wv] stationary, 128 PSUM rows).
  * Scores are plain fp8 (contraction 64): S^T tiles [128 keys, n cols] so
    P^T feeds PV directly with no transposes. DoubleRow gains nothing here
    (the PE is moving-operand-ingest-bound) and would double the stream.
  * P and V are fp8 for out rows >= 256 (those average over >= 256 keys, so
    fp8 V quantization noise cancels ~1/sqrt(n)); rows 0:255 keep P, V in
    bf16 (few-key rows see V error directly). A separate bf16 copy of k's
    first 256 rows feeds the bf16 V-head projection.
  * PV uses fp8 DoubleRow with the pair dim = two key tiles: halves the
    P^T stream. V-natural tiles come from PE transposes of V^T (kept on
    SBUF partitions 64:128 straight from the fused KV projection PSUM,
    transposed from there via tile_position=(64,0)).
  * Softmax denominators come from a ones column in the PV stationary
    (32.0 in the fp8 path to match the 32-scaled V; the scale cancels in
    the final divide). The kernel emits UNNORMALIZED [65, T] (row 0 =
    denominator); the host does rows[1:65]/row[0] on unshard.
  * exp runs on the scalar engine out of PSUM in [128, 2, n] key-tile
    pairs; the activation table is pre-warmed with a dummy exp during the
    DMA fill. The scalar engine does nothing else (exp is ~21us).
  * Engine split: PE matmuls; Act exp (plus the q-block DMA ring); DVE
    proj/PSUM copies and output staging; Pool (gpsimd) diagonal masks and
    V18 zero fill. Inputs stream on the sync + scalar HWDGE rings (one
    tile per 512-col block, blocks 0-1 split per 256-chunk, for precise
    DMA deps); the slower gpsimd ring carries the k head + bf16 consts;
    output stores ride the sync ring after the fill drains.
  * Pipeline: a single global score->exp->PV chain over all key-tile
    pairs, with PV trailing scores by three pairs so the PE never waits
    on exp or masks; projection blocks are interleaved at fixed points.
"""

import numpy as np
import ml_dtypes

B, T, C, H = 8, 2048, 1024, 64
P = 128
NB = T // 512             # 4 column blocks
NJ = T // P               # 16 key tiles
WS = 32.0                 # fp8 weight pre-scale
SCALE = float(C) ** -0.5 / (WS * WS)   # folded into the exp activation

_cached = {}


def _build():
    import concourse.bass as bass
    import concourse.mybir as mybir
    import concourse.tile as tile
    from concourse import bacc

    dt = mybir.dt
    DR = mybir.MatmulPerfMode.DoubleRow
    EXP = mybir.ActivationFunctionType.Exp
    nc = bacc.Bacc("TRN2", target_bir_lowering=False, debug=False, num_devices=B)

    # inputs (see _host_inputs for layouts)
    q8T = nc.dram_tensor("q8T", [NB, 4, P, 2, 512], dt.float8e4, kind="ExternalInput").ap()
    k8T = nc.dram_tensor("k8T", [NB, 4, P, 2, 512], dt.float8e4, kind="ExternalInput").ap()
    khT = nc.dram_tensor("khT", [P, 8, 256], dt.bfloat16, kind="ExternalInput").ap()
    wq8 = nc.dram_tensor("wq8", [P, 4, 2, H], dt.float8e4, kind="ExternalInput").ap()
    wkv8 = nc.dram_tensor("wkv8", [P, 4, 2, P], dt.float8e4, kind="ExternalInput").ap()
    wvh = nc.dram_tensor("wvh", [P, 8, H], dt.bfloat16, kind="ExternalInput").ap()
    mask8 = nc.dram_tensor("mask8", [P, 2, 256], dt.float8e4, kind="ExternalInput").ap()
    maskh = nc.dram_tensor("maskh", [P, 2, 256], dt.bfloat16, kind="ExternalInput").ap()
    idb8 = nc.dram_tensor("idb8", [H, H], dt.float8e4, kind="ExternalInput").ap()
    idbh = nc.dram_tensor("idbh", [H, H], dt.bfloat16, kind="ExternalInput").ap()
    out_t = nc.dram_tensor("out_t", [H + 1, T], dt.float32, kind="ExternalOutput").ap()

    with tile.TileContext(nc) as tc:
        with (
            tc.tile_pool(name="consts", bufs=1) as consts,
            tc.tile_pool(name="inbuf", bufs=1) as inbuf,
            tc.tile_pool(name="proj", bufs=1) as proj,
            tc.tile_pool(name="p8buf", bufs=4) as p8buf,
            tc.tile_pool(name="obuf", bufs=2) as obuf,
            tc.tile_pool(name="ppsum", bufs=2, space="PSUM") as ppsum,
            tc.tile_pool(name="vtpsum", bufs=1, space="PSUM") as vtpsum,
            tc.tile_pool(name="opsum", bufs=2, space="PSUM") as opsum,
            tc.tile_pool(name="spsum", bufs=3, space="PSUM") as spsum,
        ):
            wkv8_s = consts.tile([P, 4, 2, P], dt.float8e4)
            wq8_s = consts.tile([P, 4, 2, H], dt.float8e4)
            wvh_s = consts.tile([P, 8, H], dt.bfloat16)
            mask8_s = consts.tile([P, 2, 256], dt.float8e4)
            maskh_s = consts.tile([P, 2, 256], dt.bfloat16)
            idb8_s = consts.tile([P, H], dt.float8e4)  # identity on parts 64:128
            idbh_s = consts.tile([H, H], dt.bfloat16)

            QT8 = proj.tile([H, T], dt.float8e4)
            KT8 = proj.tile([H, T], dt.float8e4)
            VT8 = proj.tile([P, T], dt.float8e4)       # V^T on partitions 64:128
            VhT = proj.tile([H, 256], dt.bfloat16)
            V18 = proj.tile([P, NJ // 2, 2, P], dt.float8e4)   # ones(32)+V nat
            V1h = proj.tile([P, 2, 66], dt.bfloat16)           # ones(1)+Vhead
            Pth = proj.tile([P, 2, 256], dt.bfloat16)          # seg-0a P tiles

            # V18 zero fill (pad cols + unwritten regions) while DMAs run
            nc.gpsimd.memset(V18[:], 0.0)
            nc.gpsimd.memset(V18[:, :, :, 0:1], WS)
            nc.gpsimd.memset(V1h[:, :, 0:1], 1.0)

            # ---- input DMAs upfront: one tile per block for precise deps;
            # sync ring: k + small fp8 consts; scalar ring: q (then free
            # for exp); gpsimd ring (slow): k head + bf16 consts + stores.
            # blocks 0-1: a tile per 256-chunk (cc-major dram slice is one
            # contiguous 128KB DMA; consumers wait only their own chunk);
            # blocks 2-3: whole-block tiles (consumed late, slack ample).
            k8T_s = [[inbuf.tile([P, 2, 512], dt.float8e4, name=f"k8T{tb}c{cc}")
                      for cc in range(4)] if tb < 2 else
                     inbuf.tile([P, 4, 2, 512], dt.float8e4, name=f"k8T{tb}")
                     for tb in range(NB)]
            q8T_s = [[inbuf.tile([P, 2, 512], dt.float8e4, name=f"q8T{tb}c{cc}")
                      for cc in range(4)] if tb < 2 else
                     inbuf.tile([P, 4, 2, 512], dt.float8e4, name=f"q8T{tb}")
                     for tb in range(NB)]
            khT_s = inbuf.tile([P, 8, 256], dt.bfloat16)
            nc.scalar.dma_start(out=wkv8_s[:], in_=wkv8[:])
            nc.scalar.dma_start(out=wq8_s[:], in_=wq8[:])
            for cc in range(4):
                nc.scalar.dma_start(out=q8T_s[0][cc][:], in_=q8T[0, cc])
                nc.sync.dma_start(out=k8T_s[0][cc][:], in_=k8T[0, cc])
            # warm the exp activation table while the fill runs
            scr = consts.tile([1, 2], dt.float32)
            nc.gpsimd.memset(scr[:], 0.0)
            scrd = consts.tile([1, 2], dt.bfloat16)
            nc.scalar.activation(out=scrd[:], in_=scr[:], func=EXP, scale=1.0)
            nc.sync.dma_start(out=khT_s[:], in_=khT[:])
            nc.gpsimd.dma_start(out=wvh_s[:], in_=wvh[:])
            nc.gpsimd.dma_start(out=idbh_s[:], in_=idbh[:])
            nc.sync.dma_start(out=idb8_s[64:P, :], in_=idb8[:])
            for cc in range(4):
                nc.scalar.dma_start(out=q8T_s[1][cc][:], in_=q8T[1, cc])
                nc.sync.dma_start(out=k8T_s[1][cc][:], in_=k8T[1, cc])
            nc.sync.dma_start(out=maskh_s[:], in_=maskh[:])
            for cc in range(4):
                nc.scalar.dma_start(out=q8T_s[2][:, cc], in_=q8T[2, cc])
                nc.sync.dma_start(out=k8T_s[2][:, cc], in_=k8T[2, cc])
            nc.sync.dma_start(out=mask8_s[:], in_=mask8[:])
            for cc in range(4):
                nc.scalar.dma_start(out=q8T_s[3][:, cc], in_=q8T[3, cc])
                nc.sync.dma_start(out=k8T_s[3][:, cc], in_=k8T[3, cc])

            # ---- projection blocks ---------------------------------------
            def proj_block(tb):
                sl = slice(512 * tb, 512 * (tb + 1))
                QTp = ppsum.tile([H, 512], dt.float32, tag="pp")
                for cc in range(4):
                    qrhs = q8T_s[tb][cc][:] if tb < 2 else q8T_s[tb][:, cc]
                    nc.tensor.matmul(QTp[:], lhsT=wq8_s[:, cc], rhs=qrhs,
                                     start=(cc == 0), stop=(cc == 3), perf_mode=DR)
                nc.vector.tensor_copy(out=QT8[:, sl], in_=QTp[:])
                KVp = ppsum.tile([P, 512], dt.float32, tag="pp")
                for cc in range(4):
                    krhs = k8T_s[tb][cc][:] if tb < 2 else k8T_s[tb][:, cc]
                    nc.tensor.matmul(KVp[:], lhsT=wkv8_s[:, cc], rhs=krhs,
                                     start=(cc == 0), stop=(cc == 3), perf_mode=DR)
                nc.vector.tensor_copy(out=VT8[H:P, sl], in_=KVp[H:P, :])
                nc.vector.tensor_copy(out=KT8[:, sl], in_=KVp[0:H, :])
                for jj in range(4):
                    j = 4 * tb + jj
                    # fp8 PE transpose requires output element step of 2
                    vtp = vtpsum.tile([P, 2 * H], dt.float8e4, tag="vt")
                    nc.tensor.transpose(vtp[:, 0:2 * H:2],
                                        VT8[H:P, P * j:P * (j + 1)],
                                        idb8_s[H:P, :], tile_position=(H, 0))
                    nc.vector.tensor_copy(out=V18[:, j >> 1, j & 1, 1:65],
                                          in_=vtp[:, 0:2 * H:2])

            def vhead_block():
                VhTp = ppsum.tile([H, 256], dt.float32, tag="pp")
                for ch in range(8):
                    nc.tensor.matmul(VhTp[:], lhsT=wvh_s[:, ch], rhs=khT_s[:, ch],
                                     start=(ch == 0), stop=(ch == 7))
                nc.vector.tensor_copy(out=VhT[:], in_=VhTp[:])
                for j in range(2):
                    vtp = vtpsum.tile([P, H], dt.bfloat16, tag="vt")
                    nc.tensor.transpose(vtp[:], VhT[:, P * j:P * (j + 1)], idbh_s[:])
                    nc.vector.tensor_copy(out=V1h[:, j, 1:65], in_=vtp[:])

            # ---- attention work list -------------------------------------
            # segment = one OUT accumulation: (lo, w, path); its pairs:
            # (j0, n, off, diag) with off = col offset within the segment.
            segs = [(0, 256, "h", [(0, 256, 0, True)]),
                    (256, 256, "8", [(0, 256, 0, False), (2, 256, 0, True)])]
            for ic in range(1, NB):
                # masked diag pairs go before the last sub-diagonal pair so
                # the pipeline-exposed segment-final PV has no mask latency
                subs = [(2 * p, 512, 0, False) for p in range(2 * ic)]
                diags = [(4 * ic, 512, 0, True), (4 * ic + 2, 256, 256, True)]
                segs.append((512 * ic, 512, "8", subs[:-1] + diags + subs[-1:]))

            def scores(seg, pi):
                lo, w, path, prs = seg
                j0, n, off, diag = prs[pi]
                Sp = []
                for kt in range(2):
                    j = j0 + kt
                    S1 = spsum.tile([P, 512], dt.float32, tag="s",
                                    name=f"S{j}")
                    nc.tensor.matmul(S1[:, 0:n],
                                     lhsT=KT8[:, P * j:P * (j + 1)],
                                     rhs=QT8[:, lo + off:lo + off + n],
                                     start=True, stop=True)
                    Sp.append(S1)
                return Sp

            def exp_mask(seg, pi, Sp):
                lo, w, path, prs = seg
                j0, n, off, diag = prs[pi]
                if path == "h":
                    for kt in range(2):
                        nc.scalar.activation(out=Pth[:, kt, 0:n],
                                             in_=Sp[kt][:, 0:n],
                                             func=EXP, scale=SCALE)
                    nc.gpsimd.tensor_mul(Pth[:, :, 0:256], Pth[:, :, 0:256],
                                         maskh_s[:])
                    return None
                Pt = p8buf.tile([P, 2, 512], dt.float8e4, tag="p8")
                for kt in range(2):
                    nc.scalar.activation(out=Pt[:, kt, 0:n], in_=Sp[kt][:, 0:n],
                                         func=EXP, scale=SCALE)
                if diag:
                    nc.gpsimd.tensor_mul(Pt[:, :, 0:256], Pt[:, :, 0:256],
                                         mask8_s[:])
                return Pt

            outp = {}

            def pv_emit(si, pi, Pt):
                seg = segs[si]
                lo, w, path, prs = seg
                j0, n, off, diag = prs[pi]
                OUTp = outp[si]
                if path == "h":
                    for kt in range(2):
                        nc.tensor.matmul(OUTp[0:65, 0:256],
                                         lhsT=V1h[:, kt, 0:65],
                                         rhs=Pth[:, kt, 0:256],
                                         start=(kt == 0), stop=(kt == 1))
                else:
                    nc.tensor.matmul(OUTp[:, off:w], lhsT=V18[:, j0 >> 1, :, :],
                                     rhs=Pt[:, :, 0:n],
                                     start=(pi == 0), stop=(pi == len(prs) - 1),
                                     perf_mode=DR)
                if pi == len(prs) - 1:
                    ot = obuf.tile([H + 1, 512], dt.float32, tag="o",
                                   name=f"ot{si}")
                    nc.vector.tensor_copy(out=ot[:, 0:w], in_=OUTp[0:H + 1, 0:w])
                    nc.sync.dma_start(out=out_t[:, lo:lo + w], in_=ot[:, 0:w])

            work = [(si, pi) for si, seg in enumerate(segs)
                    for pi in range(len(seg[3]))]
            pending = []
            proj_hooks = {(1, 0): 1, (2, 1): 2, (3, 1): 3}
            proj_block(0)
            for si, pi in work:
                if pi == 0:
                    outp[si] = opsum.tile([P, 512], dt.float32, tag="out",
                                          name=f"OUTp{si}")
                Sp = scores(segs[si], pi)
                if (si, pi) in proj_hooks:
                    proj_block(proj_hooks[(si, pi)])
                if (si, pi) == (1, 1):
                    vhead_block()
                if len(pending) >= 3:
                    pv_emit(*pending.pop(0))
                pending.append((si, pi, exp_mask(segs[si], pi, Sp)))
            for args in pending:
                pv_emit(*args)

    nc.compile()
    return nc


def _get_nc():
    if "nc" not in _cached:
        _cached["nc"] = _build()
    return _cached["nc"]


def _block8(xT):
    """fp8 [C, T] -> [NB, 4, P, 2, 512]; c = cc*256 + kt*128 + p."""
    return np.ascontiguousarray(
        xT.reshape(4, 2, P, NB, 512).transpose(3, 0, 2, 1, 4))


def _w8(w):
    """fp8 [C, Hw] -> [P, 4, 2, Hw]."""
    return np.ascontiguousarray(
        w.reshape(4, 2, P, w.shape[1]).transpose(2, 0, 1, 3))


def _host_inputs(q, k, Wq, Wk, Wv):
    bf16 = ml_dtypes.bfloat16
    f8 = ml_dtypes.float8_e4m3
    wq8_h = _w8((WS * Wq).astype(f8))
    wkv8_h = _w8((WS * np.concatenate([Wk, Wv], axis=1)).astype(f8))
    wvh_h = np.ascontiguousarray(
        Wv.astype(bf16).reshape(8, P, H).transpose(1, 0, 2))
    tri = np.triu(np.ones((P, P), np.float32))
    m = np.zeros((P, 2, 256), np.float32)
    m[:, 0, 0:128] = tri
    m[:, 0, 128:256] = 1.0
    m[:, 1, 128:256] = tri
    idb = np.eye(H, dtype=np.float32)
    consts = {
        "wq8": wq8_h, "wkv8": wkv8_h, "wvh": wvh_h,
        "mask8": m.astype(f8), "maskh": m.astype(bf16),
        "idb8": idb.astype(f8), "idbh": idb.astype(bf16),
    }
    in_maps = []
    for b in range(B):
        in_maps.append({
            "q8T": _block8(q[b].T.astype(f8)),
            "k8T": _block8(k[b].T.astype(f8)),
            "khT": np.ascontiguousarray(
                k[b, :256].T.astype(bf16).reshape(8, P, 256).transpose(1, 0, 2)),
            **consts,
        })
    return in_maps


def _postprocess(res):
    out = np.empty((B, T, H), np.float32)
    for b in range(B):
        o = res.results[b]["out_t"]
        out[b] = (o[1:H + 1] / o[0:1]).T
    return out


def kernel(q, k, Wq, Wk, Wv):
    from concourse.bass_utils import run_bass_kernel_spmd

    nc = _get_nc()
    in_maps = _host_inputs(q, k, Wq, Wk, Wv)
    res = run_bass_kernel_spmd(nc, in_maps, list(range(B)))
    return _postprocess(res)


if __name__ == "__main__":
    rng = np.random.default_rng(0)
    q = rng.standard_normal((B, T, C)).astype(np.float32)
    k = rng.standard_normal((B, T, C)).astype(np.float32)
    Wq = (rng.standard_normal((C, H)) * 0.02).astype(np.float32)
    Wk = (rng.standard_normal((C, H)) * 0.02).astype(np.float32)
    Wv = (rng.standard_normal((C, H)) * 0.02).astype(np.float32)
    o = kernel(q, k, Wq, Wk, Wv)
    print("out", o.shape, o.dtype, float(np.abs(o).max()))
